# revision 1
# baseline (speedup 1.0000x reference)
"""Trainium2 Bass kernel for masked-pool + per-sample expert matmul (moe_routing).

Computation (reference):
    attended[b,c] = mean_hw(mask[b,hw] * features[b,c,hw])        # [B,C]
    preds[b,a]    = sum_c attended[b,c] * weight[inst[b],c,a] + bias[inst[b],a]

Sharding: expert-parallel with host-side routing. The 32 experts are packed
into 8 bins of 4 (balanced by sample count); each core gets the features of
the samples routed to its 4 experts (padded to S rows), its 4 experts'
weights, and an indicator matrix ind[slot, row] = 1/196 marking which rows
belong to which expert slot. On device, each slot's matmul uses the
indicator-masked attended matrix as the stationary operand, accumulating all
4 slots into one PSUM tile, so each core reads only its own 4 experts'
weights (16.4MB) + its own samples' features (~13MB) -- near the HBM
roofline for this memory-bound problem.
"""

import numpy as np

import concourse.bacc as bacc
import concourse.tile as tile
from concourse import mybir
from concourse.bass_utils import run_bass_kernel_spmd

B, C, H, W = 256, 512, 14, 14
HWD = H * W  # 196
N_EXP, N_ANS = 32, 2000
N_CORES = 8
E = N_EXP // N_CORES  # expert slots per core = 4
S_DEFAULT = 32        # padded samples per core (>= max balanced bin load)
J = C // 128          # c-chunks = 4
N_TILE = 512
NT = (N_ANS + N_TILE - 1) // N_TILE  # 4 (2000 = 3*512 + 464)
N_ACH = 16            # 128-wide output chunks (2000 -> 16 chunks, last = 80)
A_PAD = N_ACH * 128   # padded answer dim for the transposed output (2048)
GB = 8                # samples per feature-DMA batch

def WT_GATE_MS(t_idx):
    """Model-time gate (ms) for weight-tile DMA t_idx (scheduling hint)."""
    return 0.038 + 0.0029 * t_idx

_compiled = {}  # S -> nc
_runners = {}   # S -> callable(in_maps) -> per-core result dicts


def _make_runner(nc):
    """Build a reusable jitted SPMD executor for `nc` (jit traced once, so
    repeat kernel() calls skip retracing; mirrors bass2jax.run_bass_via_pjrt).
    """
    import jax
    from jax.experimental.shard_map import shard_map
    from jax.sharding import Mesh, PartitionSpec
    from concourse.bass2jax import (_bass_exec_p, install_neuronx_cc_hook,
                                    partition_id_tensor)

    install_neuronx_cc_hook()
    pname = nc.partition_id_tensor.name if nc.partition_id_tensor else None
    in_names, out_names, out_avals = [], [], []
    for alloc in nc.m.functions[0].allocations:
        if not isinstance(alloc, mybir.MemoryLocationSet):
            continue
        name = alloc.memorylocations[0].name
        if alloc.kind == "ExternalInput":
            if name != pname:
                in_names.append(name)
        elif alloc.kind == "ExternalOutput":
            out_names.append(name)
            out_avals.append(jax.core.ShapedArray(
                tuple(alloc.tensor_shape), mybir.dt.np(alloc.dtype)))
    n_params = len(in_names)
    n_outs = len(out_avals)
    all_in = in_names + out_names + ([pname] if pname else [])
    donate = tuple(range(n_params, n_params + n_outs))

    def _body(*args):
        operands = list(args)
        if pname is not None:
            operands.append(partition_id_tensor())
        return tuple(_bass_exec_p.bind(
            *operands, out_avals=tuple(out_avals), in_names=tuple(all_in),
            out_names=tuple(out_names), lowering_input_output_aliases=(),
            sim_require_finite=True, sim_require_nnan=True, nc=nc))

    devices = jax.devices()[:N_CORES]
    mesh = Mesh(np.asarray(devices), ("core",))
    sharded = jax.jit(
        shard_map(_body, mesh=mesh,
                  in_specs=(PartitionSpec("core"),) * (n_params + n_outs),
                  out_specs=(PartitionSpec("core"),) * n_outs,
                  check_rep=False),
        donate_argnums=donate, keep_unused=True)

    def run(in_maps):
        concat_in = [
            np.concatenate([np.asarray(m[name]) for m in in_maps], axis=0)
            for name in in_names
        ]
        zeros = [np.zeros((N_CORES * a.shape[0], *a.shape[1:]), a.dtype)
                 for a in out_avals]
        out = sharded(*concat_in, *zeros)
        return [
            {name: np.asarray(out[i]).reshape(N_CORES, *out_avals[i].shape)[c]
             for i, name in enumerate(out_names)}
            for c in range(N_CORES)
        ]

    return run


def _get_runner(S):
    if S not in _runners:
        _runners[S] = _make_runner(_get_compiled(S))
    return _runners[S]


def _build(S):
    fp32 = mybir.dt.float32
    nc = bacc.Bacc("TRN2", target_bir_lowering=False, debug=False,
                   num_devices=N_CORES)
    feat = nc.dram_tensor("feat", [S, C, HWD], fp32, kind="ExternalInput")
    maskv = nc.dram_tensor("maskv", [1, S, HWD], fp32, kind="ExternalInput")
    wt = nc.dram_tensor("wt", [E, C, N_ANS], fp32, kind="ExternalInput")
    be = nc.dram_tensor("be", [E, A_PAD], fp32, kind="ExternalInput")
    ind = nc.dram_tensor("ind", [1, E, S], fp32, kind="ExternalInput")
    ind01 = nc.dram_tensor("ind01", [E, S], fp32, kind="ExternalInput")
    outT = nc.dram_tensor("outT", [A_PAD, S], fp32, kind="ExternalOutput")

    # shrink the prefetch pools when a pathological routing forces S far
    # beyond the balanced 32 rows/core (keeps SBUF within budget; perf of
    # the fallback is secondary)
    f_bufs = 3 if S <= 48 else 2
    w_bufs = 10 if S <= 48 else (6 if S <= 128 else 3)
    with tile.TileContext(nc) as tc:
        with (
            tc.tile_pool(name="persist", bufs=1) as persist,
            tc.tile_pool(name="fpool", bufs=f_bufs) as fpool,
            tc.tile_pool(name="mrpool", bufs=2) as mrpool,
            tc.tile_pool(name="mpool", bufs=2) as mpool,
            tc.tile_pool(name="ppool", bufs=3) as ppool,
            tc.tile_pool(name="spool", bufs=2) as spool,
            tc.tile_pool(name="wpool", bufs=w_bufs) as wpool,
            tc.tile_pool(name="psum", bufs=4, space="PSUM") as psum_pool,
        ):
            attT = persist.tile([128, J, S], fp32)   # attended^T (unscaled)
            # phase 1: attT[c,j,i] = sum_hw feat[i, j*128+c, hw] * mask[i,hw]
            # work is spread over three engines per sample: DVE multiplies
            # chunks 0-1 and reduces chunks 2-3; Pool multiplies chunks 2-3;
            # ACT reduces chunks 0-1.
            for i0 in range(0, S, GB):
                g = min(GB, S - i0)
                mrow = mrpool.tile([1, GB, HWD], fp32, tag="mr")
                nc.sync.dma_start(mrow[:, :g], maskv.ap()[:, i0:i0 + g])
                ft = fpool.tile([128, GB, J, HWD], fp32, tag="ft")
                nc.sync.dma_start(
                    ft[:, :g],
                    feat.ap()[i0:i0 + g].rearrange("s (j p) h -> p s j h",
                                                   p=128))
                mb = mpool.tile([128, GB, HWD], fp32, tag="mb")
                nc.gpsimd.partition_broadcast(
                    mb[:, :g, :], mrow[:, :g, :])
                for s in range(g):
                    i = i0 + s
                    pr01 = ppool.tile([128, 2, HWD], fp32, tag="pr01")
                    pr23 = ppool.tile([128, 2, HWD], fp32, tag="pr23")
                    mbb2 = mb[:, s, None, :].to_broadcast((128, 2, HWD))
                    nc.vector.tensor_mul(pr01[:], ft[:, s, 0:2, :], mbb2)
                    nc.gpsimd.tensor_mul(pr23[:], ft[:, s, 2:4, :], mbb2)
                    for j in range(2):
                        scr = spool.tile([128, HWD], fp32, tag="scr")
                        nc.scalar.activation(
                            scr[:], pr01[:, j, :],
                            mybir.ActivationFunctionType.Copy,
                            accum_out=attT[:, j, i:i + 1])
                    nc.vector.tensor_reduce(
                        attT[:, 2:4, i:i + 1], pr23[:],
                        mybir.AxisListType.X, mybir.AluOpType.add)

            indb = persist.tile([128, E, S], fp32)
            nc.sync.dma_start(indb[:], ind.ap().to_broadcast((128, E, S)))
            be_sb = persist.tile([E, A_PAD], fp32)
            nc.sync.dma_start(be_sb[:], be.ap())
            i01_sb = persist.tile([E, S], fp32)
            nc.sync.dma_start(i01_sb[:], ind01.ap())

            # indicator mask (also folds in the 1/196 mean scaling)
            matt = persist.tile([128, E, J, S], fp32)
            for g in range(E):
                for j in range(J):
                    nc.vector.tensor_mul(
                        matt[:, g, j, :], attT[:, j, :], indb[:, g, :])

            # phase 2 (output transposed: psum[a,s] so the PE streams the
            # small matt operand, keeping full fp32 at ~4x less PE time):
            # outT[a,i] = sum_g sum_c wt[g,c,a] * matt[c,g,i] + bias
            # where bias arrives in PSUM via a K=4 matmul be.T @ ind01.
            out_sbT = persist.tile([128, N_ACH, S], fp32)
            if N_ANS % 128:
                # rows beyond N_ANS in the last chunk are never computed;
                # zero them so the padded outT DMA reads initialized data
                lo = (N_ANS % 128) // 32 * 32
                nc.vector.memset(out_sbT[lo:, N_ACH - 1, :], 0.0)
            for nt in range(NT):
                n0 = nt * N_TILE
                n1 = min(N_ANS, n0 + N_TILE)
                wt_tiles = []
                for g in range(E):
                    t_idx = nt * E + g
                    wtile = wpool.tile([128, J, N_TILE], fp32, tag="wt")
                    # stagger weight fetches behind the feature stream so
                    # phase 1 is never starved of DMA bandwidth
                    with tc.tile_wait_until(WT_GATE_MS(t_idx)):
                        nc.sync.dma_start(
                            wtile[:, :, :n1 - n0],
                            wt.ap()[g, :, n0:n1].rearrange(
                                "(j p) a -> p j a", p=128))
                    wt_tiles.append(wtile)
                n_ac = (n1 - n0 + 127) // 128
                for ac in range(n_ac):
                    a0 = ac * 128
                    w = min(128, n1 - n0 - a0)
                    acg = nt * 4 + ac
                    ps = psum_pool.tile([128, S], fp32, tag="ps")
                    # bias first (start=True zero-initializes the region) so
                    # it is off the critical path after the last weight tile
                    nc.tensor.matmul(
                        ps[:w, :],
                        be_sb[:, n0 + a0:n0 + a0 + w],
                        i01_sb[:],
                        start=True, stop=False)
                    k = 0
                    for g in range(E):
                        for j in range(J):
                            nc.tensor.matmul(
                                ps[:w, :],
                                wt_tiles[g][:, j, a0:a0 + w],
                                matt[:, g, j, :],
                                start=False, stop=(k == E * J - 1))
                            k += 1
                    nc.vector.tensor_copy(out_sbT[:w, acg, :], ps[:w, :])
                    nc.scalar.dma_start(
                        outT.ap()[acg * 128:(acg + 1) * 128]
                        .rearrange("(q p) s -> p q s", p=128),
                        out_sbT[:, acg:acg + 1, :])
    nc.compile()
    return nc


def _get_compiled(S):
    if S not in _compiled:
        _compiled[S] = _build(S)
    return _compiled[S]


def _exact_partition(cnt, cap):
    """Try to split the 32 experts into 8 groups of 4 with group-sum <= cap.

    Builds groups one at a time: each group takes the largest remaining
    expert plus 3 companions chosen by DFS over distinct count-combinations.
    Returns bins (list of expert-id groups) or None.
    """
    import itertools

    budget = [500000]

    def solve(ids):
        if not ids:
            return []
        if budget[0] <= 0:
            return None
        ids = sorted(ids, key=lambda e: -cnt[e])
        first = ids[0]
        rest = ids[1:]
        n = len(rest)
        seen = set()
        for combo in itertools.combinations(range(n), E - 1):
            budget[0] -= 1
            if budget[0] <= 0:
                return None
            vals = tuple(cnt[rest[i]] for i in combo)
            if cnt[first] + sum(vals) > cap or vals in seen:
                continue
            seen.add(vals)
            remaining = [rest[i] for i in range(n) if i not in combo]
            sub = solve(remaining)
            if sub is not None:
                return [[first] + [rest[i] for i in combo]] + sub
        return None

    return solve(list(range(N_EXP)))


def _route(instance):
    """Pack 32 experts into 8 bins of 4, balanced by sample count.

    Returns (bins, sample_lists, max_load): bins[c] = 4 expert ids,
    sample_lists[c] = sample indices routed to core c (grouped by expert).
    """
    cnt = np.bincount(instance, minlength=N_EXP)
    # perfect balance first: groups of 4 experts each with <= ceil(B/8)
    cap = (int(cnt.sum()) + N_CORES - 1) // N_CORES
    bins = _exact_partition(cnt, cap)
    if bins is None:
        order = np.argsort(-cnt, kind="stable")
        bins = [[] for _ in range(N_CORES)]
        loads = [0] * N_CORES
        for e in order:
            cands = [b for b in range(N_CORES) if len(bins[b]) < E]
            b = min(cands, key=lambda x: loads[x])
            bins[b].append(int(e))
            loads[b] += int(cnt[e])
    sample_lists = [
        np.concatenate([np.where(instance == e)[0] for e in bins[c]])
        for c in range(N_CORES)
    ]
    return bins, sample_lists, max(len(s) for s in sample_lists)


def make_in_maps(mask, features, weight, bias, inst, S, bins, sample_lists):
    feat_flat = features.reshape(B, C, HWD)
    mask_flat = mask.reshape(B, HWD)
    in_maps = []
    for c in range(N_CORES):
        samp = sample_lists[c]
        n_c = len(samp)
        if n_c > 0:
            padded = np.concatenate([samp, np.full(S - n_c, samp[0])])
        else:
            padded = np.zeros(S, dtype=np.int64)
        ind_c = np.zeros((1, E, S), dtype=np.float32)
        slot_of = {e: g for g, e in enumerate(bins[c])}
        for k in range(n_c):
            ind_c[0, slot_of[int(inst[samp[k]])], k] = 1.0 / HWD
        be_c = np.zeros((E, A_PAD), dtype=np.float32)
        be_c[:, :N_ANS] = bias[bins[c]]
        ind01_c = (ind_c[0] != 0).astype(np.float32)
        in_maps.append({
            "feat": np.ascontiguousarray(feat_flat[padded]),
            "maskv": np.ascontiguousarray(mask_flat[padded])[None],
            "wt": np.ascontiguousarray(weight[bins[c]]),
            "be": be_c,
            "ind": ind_c,
            "ind01": ind01_c,
        })
    return in_maps


def kernel(mask, features, weight, bias, instance):
    mask = np.ascontiguousarray(np.asarray(mask, dtype=np.float32))
    features = np.ascontiguousarray(np.asarray(features, dtype=np.float32))
    weight = np.ascontiguousarray(np.asarray(weight, dtype=np.float32))
    bias = np.ascontiguousarray(np.asarray(bias, dtype=np.float32))
    inst = np.asarray(instance).astype(np.int64)
    assert features.shape == (B, C, H, W)

    bins, sample_lists, max_load = _route(inst)
    S = max(S_DEFAULT, max_load)
    nc = _get_compiled(S)

    in_maps = make_in_maps(mask, features, weight, bias, inst, S, bins,
                           sample_lists)
    try:
        results = _get_runner(S)(in_maps)
    except Exception:
        results = run_bass_kernel_spmd(
            nc, in_maps, list(range(N_CORES))).results

    preds = np.empty((B, N_ANS), dtype=np.float32)
    for c in range(N_CORES):
        samp = sample_lists[c]
        preds[samp] = results[c]["outT"][:N_ANS, :len(samp)].T
    return preds


# Precompile the default-size program at import so a timed first call does
# not pay the (one-time) build+compile cost.
_get_compiled(S_DEFAULT)



# revision 2
# speedup vs baseline: 1.9080x; 1.9080x over previous
"""Trainium2 Bass kernel for masked-pool + per-sample expert matmul (moe_routing).

Computation (reference):
    attended[b,c] = mean_hw(mask[b,hw] * features[b,c,hw])        # [B,C]
    preds[b,a]    = sum_c attended[b,c] * weight[inst[b],c,a] + bias[inst[b],a]

Sharding: expert-parallel with host-side routing. The 32 experts are packed
into 8 bins of 4 (balanced by sample count); each core gets the features of
the samples routed to its 4 experts (padded to S rows), its 4 experts'
weights, and a mask tensor mtg[hw, slot, s] = mask[s,hw]*ind01[slot,s]/196.

All device tensors are fp16 (halves HBM traffic vs fp32; quantization error
~1e-4, far under the 2e-2 gate). Both phases run on the PE:
  phase 1: per (sample, c-block): matt[c, j, s, g] = sum_hw ft[hw, c] * mtg[hw, s, g]
           with hw on the partition (contraction) dim, accumulating the two
           98-row hw chunks in PSUM. The indicator in mtg masks each sample
           into its expert slot's column so phase 2 can accumulate all 4
           slots into one PSUM tile.
  phase 2: outT[a, s] = sum_g sum_c wt[g, c, a] * matt[c, s, g] + bias (via a
           K=4 matmul be.T @ ind01), output transposed [a_chunk, s].

Host-side packing gives every large DMA >=512B contiguous runs (full DMA
rate): features [98, 2, S, C], weights [E, 128, A, J], output [2, 128, 8, S].
Per-core traffic ~14.9MB -> ~41.5us at the 360B/ns DMA roofline.
"""

import numpy as np

import concourse.bacc as bacc
import concourse.tile as tile
from concourse import mybir
from concourse.bass_utils import run_bass_kernel_spmd

B, C, H, W = 256, 512, 14, 14
HWD = H * W  # 196
P = 98                # hw-partition chunk (196 = 2*98)
N_EXP, N_ANS = 32, 2000
N_CORES = 8
E = N_EXP // N_CORES  # expert slots per core = 4
S_DEFAULT = 32        # padded samples per core (>= max balanced bin load)
J = C // 128          # c-chunks = 4
NT_W = [512, 512, 512, 256, 208]  # answer-tile widths (sum = 2000); small
                                  # last tile keeps the dependent tail short
N_ACH = 16            # 128-wide output chunks (2000 -> 16 chunks, last = 80)
A_PAD = N_ACH * 128   # padded answer dim for the transposed output (2048)
GB = 8                # samples per feature-DMA batch

_compiled = {}  # S -> nc
_runners = {}   # S -> callable(in_maps) -> per-core result dicts


def _make_runner(nc):
    """Build a reusable jitted SPMD executor for `nc` (jit traced once, so
    repeat kernel() calls skip retracing; mirrors bass2jax.run_bass_via_pjrt).
    """
    import jax
    from jax.experimental.shard_map import shard_map
    from jax.sharding import Mesh, PartitionSpec
    from concourse.bass2jax import (_bass_exec_p, install_neuronx_cc_hook,
                                    partition_id_tensor)

    install_neuronx_cc_hook()
    pname = nc.partition_id_tensor.name if nc.partition_id_tensor else None
    in_names, out_names, out_avals = [], [], []
    for alloc in nc.m.functions[0].allocations:
        if not isinstance(alloc, mybir.MemoryLocationSet):
            continue
        name = alloc.memorylocations[0].name
        if alloc.kind == "ExternalInput":
            if name != pname:
                in_names.append(name)
        elif alloc.kind == "ExternalOutput":
            out_names.append(name)
            out_avals.append(jax.core.ShapedArray(
                tuple(alloc.tensor_shape), mybir.dt.np(alloc.dtype)))
    n_params = len(in_names)
    n_outs = len(out_avals)
    all_in = in_names + out_names + ([pname] if pname else [])
    donate = tuple(range(n_params, n_params + n_outs))

    def _body(*args):
        operands = list(args)
        if pname is not None:
            operands.append(partition_id_tensor())
        return tuple(_bass_exec_p.bind(
            *operands, out_avals=tuple(out_avals), in_names=tuple(all_in),
            out_names=tuple(out_names), lowering_input_output_aliases=(),
            sim_require_finite=True, sim_require_nnan=True, nc=nc))

    devices = jax.devices()[:N_CORES]
    mesh = Mesh(np.asarray(devices), ("core",))
    sharded = jax.jit(
        shard_map(_body, mesh=mesh,
                  in_specs=(PartitionSpec("core"),) * (n_params + n_outs),
                  out_specs=(PartitionSpec("core"),) * n_outs,
                  check_rep=False),
        donate_argnums=donate, keep_unused=True)

    def run(in_maps):
        concat_in = [
            np.concatenate([np.asarray(m[name]) for m in in_maps], axis=0)
            for name in in_names
        ]
        zeros = [np.zeros((N_CORES * a.shape[0], *a.shape[1:]), a.dtype)
                 for a in out_avals]
        out = sharded(*concat_in, *zeros)
        return [
            {name: np.asarray(out[i]).reshape(N_CORES, *out_avals[i].shape)[c]
             for i, name in enumerate(out_names)}
            for c in range(N_CORES)
        ]

    return run


def _get_runner(S):
    if S not in _runners:
        _runners[S] = _make_runner(_get_compiled(S))
    return _runners[S]


def _build(S):
    fp16 = mybir.dt.float16
    fp32 = mybir.dt.float32
    nc = bacc.Bacc("TRN2", target_bir_lowering=False, debug=False,
                   num_devices=N_CORES)
    ft = nc.dram_tensor("ft", [P, 2, S, C], fp16, kind="ExternalInput")
    mtg = nc.dram_tensor("mtg", [P, 2, S, E], fp16, kind="ExternalInput")
    wtd = nc.dram_tensor("wtd", [E, 128, N_ANS, J], fp16,
                         kind="ExternalInput")
    be = nc.dram_tensor("be", [E, A_PAD], fp16, kind="ExternalInput")
    i01 = nc.dram_tensor("i01", [E, S], fp16, kind="ExternalInput")
    outd = nc.dram_tensor("outd", [2, 128, N_ACH // 2, S], fp16,
                          kind="ExternalOutput")

    n_batches = (S + GB - 1) // GB
    # model-time (ms) gates for the weight-tile DMAs: hold them behind the
    # feature stream so phase 1 is never starved of DMA bandwidth. The gate
    # approximates each tile's natural start time minus the ~1.4us issue
    # pipeline (a slightly-early gate only queues the transfer, it cannot
    # preempt already-queued feature DMAs).
    ft_bytes = P * 2 * GB * C * 2
    wt_gate = []
    t_us = 1.5 + n_batches * (ft_bytes / 360.0) / 1000.0
    for w_nt in NT_W:
        for g in range(E):
            wt_gate.append(t_us / 1000.0)
            t_us += (128 * w_nt * J * 2 / 360.0) / 1000.0

    f_bufs = 3 if S <= 48 else 2
    w_bufs = 6 if S <= 128 else 3
    with tile.TileContext(nc) as tc:
        with (
            tc.tile_pool(name="persist", bufs=1) as persist,
            tc.tile_pool(name="fpool", bufs=f_bufs) as fpool,
            tc.tile_pool(name="wpool", bufs=w_bufs) as wpool,
            tc.tile_pool(name="mpsum", bufs=2, space="PSUM") as mpsum,
            tc.tile_pool(name="psum", bufs=4, space="PSUM") as psum_pool,
        ):
            mtg_sb = persist.tile([P, 2, S, E], fp16)
            be_sb = persist.tile([E, A_PAD], fp16)
            i01_sb = persist.tile([E, S], fp16)
            matt_sb = persist.tile([128, J, S, E], fp16)
            out_sb = persist.tile([128, N_ACH, S], fp16)
            if N_ANS % 128:
                # rows beyond N_ANS in the last chunk are never computed;
                # zero them so the outT DMA reads initialized data
                lo = (N_ANS % 128) // 32 * 32
                nc.vector.memset(out_sb[lo:, N_ACH - 1, :], 0.0)

            # phase 1 on PE: matt[c,j,s,g] = sum_hw ft[hw,c] * mtg[hw,s,g],
            # contracting hw (2 chunks of 98 partitions) in PSUM. The
            # stationary operand is the feature block (LdWeights), the moving
            # operand the 4 slot-masked mask columns for that sample.
            first = True
            for b in range(n_batches):
                b0 = b * GB
                g_n = min(GB, S - b0)
                ftt = fpool.tile([P, 2, GB, C], fp16, tag="ftt")
                nc.sync.dma_start(ftt[:, :, :g_n], ft.ap()[:, :, b0:b0 + g_n])
                if first:
                    # small persistent loads slot in behind the first feature
                    # batch (issue order = program order on the sync queue)
                    nc.sync.dma_start(mtg_sb[:], mtg.ap())
                    nc.sync.dma_start(be_sb[:], be.ap())
                    nc.sync.dma_start(i01_sb[:], i01.ap())
                    first = False
                mps = mpsum.tile([128, J, GB, E], fp32, tag="mps")
                for sl in range(g_n):
                    i = b0 + sl
                    for j in range(J):
                        for u in range(2):
                            nc.tensor.matmul(
                                mps[:, j, sl, :],
                                ftt[:, u, sl, j * 128:(j + 1) * 128],
                                mtg_sb[:, u, i, :],
                                start=(u == 0), stop=(u == 1))
                nc.vector.tensor_copy(matt_sb[:, :, b0:b0 + g_n, :],
                                      mps[:, :, :g_n, :])

            # phase 2 (output transposed: psum[a,s]): outT[a,i] =
            # sum_g sum_c wt[g,c,a] * matt[c,i,g] + bias, bias arriving in
            # PSUM via a K=4 matmul be.T @ ind01 (start=True zero-init).
            t_idx = 0
            n0 = 0
            acg = 0
            for nt, w_nt in enumerate(NT_W):
                wt_tiles = []
                for g in range(E):
                    wt_t = wpool.tile([128, 512, J], fp16, tag="wt")
                    with tc.tile_wait_until(wt_gate[t_idx]):
                        nc.sync.dma_start(wt_t[:, :w_nt],
                                          wtd.ap()[g, :, n0:n0 + w_nt])
                    wt_tiles.append(wt_t)
                    t_idx += 1
                for a0 in range(0, w_nt, 128):
                    w = min(128, w_nt - a0)
                    ps = psum_pool.tile([128, S], fp32, tag="ps")
                    nc.tensor.matmul(
                        ps[:w], be_sb[:, acg * 128:acg * 128 + w], i01_sb[:],
                        start=True, stop=False)
                    k = 0
                    for g in range(E):
                        for j in range(J):
                            nc.tensor.matmul(
                                ps[:w], wt_tiles[g][:, a0:a0 + w, j],
                                matt_sb[:, j, :, g],
                                start=False, stop=(k == E * J - 1))
                            k += 1
                    nc.vector.tensor_copy(out_sb[:w, acg, :], ps[:w])
                    acg += 1
                    if acg % (N_ACH // 2) == 0:
                        o = acg // (N_ACH // 2) - 1
                        nc.scalar.dma_start(
                            outd.ap()[o],
                            out_sb[:, o * (N_ACH // 2):acg, :])
                n0 += w_nt
    nc.compile()
    return nc


def _get_compiled(S):
    if S not in _compiled:
        _compiled[S] = _build(S)
    return _compiled[S]


def _exact_partition(cnt, cap):
    """Try to split the 32 experts into 8 groups of 4 with group-sum <= cap.

    Builds groups one at a time: each group takes the largest remaining
    expert plus 3 companions chosen by DFS over distinct count-combinations.
    Returns bins (list of expert-id groups) or None.
    """
    import itertools

    budget = [500000]

    def solve(ids):
        if not ids:
            return []
        if budget[0] <= 0:
            return None
        ids = sorted(ids, key=lambda e: -cnt[e])
        first = ids[0]
        rest = ids[1:]
        n = len(rest)
        seen = set()
        for combo in itertools.combinations(range(n), E - 1):
            budget[0] -= 1
            if budget[0] <= 0:
                return None
            vals = tuple(cnt[rest[i]] for i in combo)
            if cnt[first] + sum(vals) > cap or vals in seen:
                continue
            seen.add(vals)
            remaining = [rest[i] for i in range(n) if i not in combo]
            sub = solve(remaining)
            if sub is not None:
                return [[first] + [rest[i] for i in combo]] + sub
        return None

    return solve(list(range(N_EXP)))


def _route(instance):
    """Pack 32 experts into 8 bins of 4, balanced by sample count.

    Returns (bins, sample_lists, max_load): bins[c] = 4 expert ids,
    sample_lists[c] = sample indices routed to core c (grouped by expert).
    """
    cnt = np.bincount(instance, minlength=N_EXP)
    # perfect balance first: groups of 4 experts each with <= ceil(B/8)
    cap = (int(cnt.sum()) + N_CORES - 1) // N_CORES
    bins = _exact_partition(cnt, cap)
    if bins is None:
        order = np.argsort(-cnt, kind="stable")
        bins = [[] for _ in range(N_CORES)]
        loads = [0] * N_CORES
        for e in order:
            cands = [b for b in range(N_CORES) if len(bins[b]) < E]
            b = min(cands, key=lambda x: loads[x])
            bins[b].append(int(e))
            loads[b] += int(cnt[e])
    sample_lists = [
        np.concatenate([np.where(instance == e)[0] for e in bins[c]])
        for c in range(N_CORES)
    ]
    return bins, sample_lists, max(len(s) for s in sample_lists)


def make_in_maps(mask, features, weight, bias, inst, S, bins, sample_lists):
    feat_flat = features.reshape(B, C, HWD)
    mask_flat = mask.reshape(B, HWD)
    in_maps = []
    for c in range(N_CORES):
        samp = sample_lists[c]
        n_c = len(samp)
        if n_c > 0:
            padded = np.concatenate([samp, np.full(S - n_c, samp[0])])
        else:
            padded = np.zeros(S, dtype=np.int64)
        ind01 = np.zeros((E, S), dtype=np.float32)
        slot_of = {e: g for g, e in enumerate(bins[c])}
        for k in range(n_c):
            ind01[slot_of[int(inst[samp[k]])], k] = 1.0
        # features hw-major: ft[p, u, s, c] = feat[samp[s], c, u*98 + p]
        ft_c = feat_flat[padded].reshape(S, C, 2, P).transpose(3, 2, 0, 1)
        # mask, slot-masked and mean-scaled: mtg[p, u, s, g]
        mm = (mask_flat[padded] / HWD).reshape(S, 2, P).transpose(2, 1, 0)
        mtg_c = mm[:, :, :, None] * ind01.T[None, None]
        # weights a-major with c-chunk (j) innermost: wtd[g, p, a, j]
        wt_c = weight[bins[c]].reshape(E, J, 128, N_ANS).transpose(0, 2, 3, 1)
        be_c = np.zeros((E, A_PAD), dtype=np.float16)
        be_c[:, :N_ANS] = bias[bins[c]].astype(np.float16)
        in_maps.append({
            "ft": np.ascontiguousarray(ft_c, dtype=np.float16),
            "mtg": np.ascontiguousarray(mtg_c, dtype=np.float16),
            "wtd": np.ascontiguousarray(wt_c, dtype=np.float16),
            "be": be_c,
            "i01": ind01.astype(np.float16),
        })
    return in_maps


def kernel(mask, features, weight, bias, instance):
    mask = np.ascontiguousarray(np.asarray(mask, dtype=np.float32))
    features = np.ascontiguousarray(np.asarray(features, dtype=np.float32))
    weight = np.ascontiguousarray(np.asarray(weight, dtype=np.float32))
    bias = np.ascontiguousarray(np.asarray(bias, dtype=np.float32))
    inst = np.asarray(instance).astype(np.int64)
    assert features.shape == (B, C, H, W)

    bins, sample_lists, max_load = _route(inst)
    S = max(S_DEFAULT, max_load)
    nc = _get_compiled(S)

    in_maps = make_in_maps(mask, features, weight, bias, inst, S, bins,
                           sample_lists)
    try:
        results = _get_runner(S)(in_maps)
    except Exception:
        results = run_bass_kernel_spmd(
            nc, in_maps, list(range(N_CORES))).results

    preds = np.empty((B, N_ANS), dtype=np.float32)
    for c in range(N_CORES):
        samp = sample_lists[c]
        outT = results[c]["outd"].transpose(0, 2, 1, 3).reshape(A_PAD, S)
        preds[samp] = outT[:N_ANS, :len(samp)].T.astype(np.float32)
    return preds


# Precompile the default-size program at import so a timed first call does
# not pay the (one-time) build+compile cost.
_get_compiled(S_DEFAULT)
